# revision 1
# baseline (speedup 1.0000x reference)
"""Pixel-shuffle (sub-pixel conv, r=2) Trainium2 kernel.

Full op: in [16, 256, 256, 64] f32 -> out [16, 512, 512, 16] f32 with
    out[b, x, y, c] = in[b, x//2, y//2, 32*(y%2) + 16*(x%2) + c]

Sharding: batch-parallel across 8 NeuronCores (2 batches per core), no
cross-core communication.

Per-core dataflow (pure data movement; memory-bound):
  - The op is a stride-2 de-interleave of 64-byte chunks: viewing one input
    row in[b, h, :, :] as 1024 chunks of 16 floats, the even chunks form
    output row 2h and the odd chunks form output row 2h+1 (order preserved).
  - A direct DRAM->DRAM DMA would degenerate to 64 B descriptors, so instead:
    contiguous 4 MB HBM->SBUF loads, a DVE tensor_copy per output-row parity
    doing the chunk de-interleave in the free dimension, then SBUF->HBM
    stores whose DRAM-side runs are 16 KB contiguous.
  - Load and store descriptors are both 16 KB so the SDMA engines'
    packet-granular round-robin gives the two queues equal service and the
    store stream doesn't trail the loads.
"""

import numpy as np

import concourse.bass as bass
import concourse.bacc as bacc
import concourse.mybir as mybir
from concourse.tile import TileContext

# Problem shape (hardcoded; kernel.py must be self-contained).
B, H, W, CRR = 16, 256, 256, 64
R = 2
C = CRR // (R * R)  # 16
N_CORES = 8
BP = B // N_CORES  # batches per core = 2

ROWS = 64                      # input rows per tile (tile = 4 MB)
N_TILES = H // ROWS            # row-groups per batch = 4
FD = ROWS * W * CRR // 128     # SBUF free-dim floats per partition = 8192
HFD = FD // 2                  # floats per parity = 4096


def build_bass() -> bass.Bass:
    nc = bacc.Bacc()
    tin = nc.dram_tensor("t", [BP, H, W, CRR], mybir.dt.float32, kind="ExternalInput")
    tout = nc.dram_tensor(
        "out", [BP, H * R, W * R, C], mybir.dt.float32, kind="ExternalOutput"
    )

    with TileContext(nc) as tc:
        with (
            tc.tile_pool(name="src", bufs=3) as srcp,
            tc.tile_pool(name="dst", bufs=3) as dstp,
        ):
            # Prologue: the very first tile as four 1 MB sub-tiles so the
            # first store issues ~14 us earlier (shares the pools' slots).
            SROWS = 16
            for s in range(ROWS // SROWS):
                src = srcp.tile([128, SROWS * W * CRR // 128], mybir.dt.float32)
                sfd = SROWS * W * CRR // 128     # 2048
                shfd = sfd // 2                  # per-parity floats = 1024
                in_view = (
                    tin[0, s * SROWS : (s + 1) * SROWS]
                    .rearrange("h w c -> (h w c)")
                    .rearrange("(p f) -> p f", p=128)
                )
                nc.sync.dma_start(out=src[:, :], in_=in_view)
                dst = dstp.tile([128, sfd], mybir.dt.float32)
                s4 = src[:, :].rearrange("p (m i c) -> p i m c", i=R, c=C)
                for i in range(R):
                    d3 = dst[:, i * shfd : (i + 1) * shfd].rearrange(
                        "p (m c) -> p m c", c=C
                    )
                    nc.vector.tensor_copy(out=d3, in_=s4[:, i])
                # partition p = (hl in [0,16), e in [0,8)): w in [32e, 32e+32)
                # -> out rows x = 2*(s*SROWS+hl)+i, y in [64e, 64e+64).
                x0 = s * SROWS * R
                for i in range(R):
                    out_view = tout[0, x0 + i : x0 + SROWS * R : R].rearrange(
                        "hl (e m) c -> hl e (m c)", e=8
                    )
                    nc.scalar.dma_start(
                        out=out_view, in_=dst[:, i * shfd : (i + 1) * shfd]
                    )

            for b in range(BP):
                for hg in range(N_TILES):
                    if b == 0 and hg == 0:
                        continue  # handled by the prologue above
                    # ---- load: contiguous 4 MB, 32 KB per partition ----
                    # partition p = (hl, half): input row h = hg*ROWS + p//2,
                    # half = p%2 covers w in [128*half, 128*half+128); free
                    # layout in a partition: (w_local, j, i, c)
                    #   f = 64*w_local + 32*j + 16*i + c.
                    src = srcp.tile([128, FD], mybir.dt.float32)
                    in_view = (
                        tin[b, hg * ROWS : (hg + 1) * ROWS]
                        .rearrange("h w c -> (h w c)")
                        .rearrange("(p f) -> p f", p=128)
                    )
                    # Two DMAs of [128, HFD] -> 16 KB descriptors, matching the
                    # stores so SDMA round-robin serves both queues equally.
                    for h in range(2):
                        nc.sync.dma_start(
                            out=src[:, h * HFD : (h + 1) * HFD],
                            in_=in_view[:, h * HFD : (h + 1) * HFD],
                        )

                    # ---- shuffle: de-interleave 16-float chunks on DVE ----
                    # dst[p, i*HFD + m*16 + c] = src[p, m*32 + i*16 + c]
                    # (m = 2*w_local + j = output y position 256*half + m)
                    # One copy per parity so store i can start after copy i.
                    dst = dstp.tile([128, FD], mybir.dt.float32)
                    s4 = src[:, :].rearrange("p (m i c) -> p i m c", i=R, c=C)
                    for i in range(R):
                        d3 = dst[:, i * HFD : (i + 1) * HFD].rearrange(
                            "p (m c) -> p m c", c=C
                        )
                        nc.vector.tensor_copy(out=d3, in_=s4[:, i])

                    # ---- stores: one per parity, 16 KB DRAM runs ----
                    # out[b, 2*(hg*ROWS+hl)+i, 256*half + m, c]
                    #   <- dst[(hl,half), i*HFD + m*16 + c]
                    x0 = hg * ROWS * R
                    for i in range(R):
                        out_view = tout[b, x0 + i : x0 + ROWS * R : R].rearrange(
                            "hl (half m) c -> hl half (m c)", half=2
                        )
                        nc.scalar.dma_start(
                            out=out_view, in_=dst[:, i * HFD : (i + 1) * HFD]
                        )

    nc.finalize()
    return nc


_CACHE: dict[str, bass.Bass] = {}


def _get_nc() -> bass.Bass:
    if "nc" not in _CACHE:
        _CACHE["nc"] = build_bass()
    return _CACHE["nc"]


def kernel(t: np.ndarray) -> np.ndarray:
    from concourse.bass_utils import run_bass_kernel_spmd

    t = np.ascontiguousarray(np.asarray(t, dtype=np.float32))
    assert t.shape == (B, H, W, CRR), t.shape

    nc = _get_nc()
    in_maps = [{"t": t[i * BP : (i + 1) * BP]} for i in range(N_CORES)]
    res = run_bass_kernel_spmd(nc, in_maps, list(range(N_CORES)))
    return np.concatenate([r["out"] for r in res.results], axis=0)

